# revision 10
# baseline (speedup 1.0000x reference)
"""Trainium2 Bass kernel for ConditionalMoEWithLoadBalancing.

Sharding: token-parallel SPMD over 8 NeuronCores. Core c processes tokens
[c*512, (c+1)*512) through the full network (router + difficulty net in exact
fp32, all 8 expert FFNs in float32r). Expert weights are streamed from HBM
per expert; the weighted combine happens on-chip, so the host only
concatenates the 8 output shards and sums the 8 usage partials for the cv
scalar.

Self-contained: hardcodes all shapes for the fixed problem instance
(x (2,2048,1024), E=8, D=1024).
"""

import numpy as np

# problem dims
B, N, D = 2, 2048, 1024
T = B * N            # 4096 tokens
E = 8
F = 2 * D            # 2048
DD = D // 2          # 512
NC = 8               # cores
TS = T // NC         # 512 tokens per core
KX = D // 128 + 1    # 9 k-tiles for x (aug bias row)
KF = F // 128 + 1    # 17 k-tiles for h1 (aug bias row)
MIN_E, MAX_E = 1, 8
T_LOW, T_HIGH = 0.5, 2.0

_prog_cache = {}


def _build_program():
    import concourse.tile as tile
    import concourse.mybir as mybir
    from concourse import bacc
    from contextlib import ExitStack

    f32 = mybir.dt.float32
    f32r = mybir.dt.float32r
    i32 = mybir.dt.int32
    AF = mybir.ActivationFunctionType
    ALU = mybir.AluOpType
    AX = mybir.AxisListType

    nc = bacc.Bacc(
        "TRN2",
        target_bir_lowering=False,
        debug=False,
        enable_asserts=False,
        num_devices=NC,
    )

    # ---- DRAM I/O ----
    xs_d = nc.dram_tensor("xs", (128, KX, TS), f32, kind="ExternalInput").ap()
    w1_d = nc.dram_tensor("w1", (E, D, F), f32r, kind="ExternalInput").ap()
    w2_d = nc.dram_tensor("w2a", (E, KF * 128, D), f32r, kind="ExternalInput").ap()
    wd1_d = nc.dram_tensor("wd1a", (128, KX, DD), f32, kind="ExternalInput").ap()
    wg_d = nc.dram_tensor("wga", (128, KX, E), f32, kind="ExternalInput").ap()
    b1_d = nc.dram_tensor("b1p", (128, E, F // 128), f32, kind="ExternalInput").ap()
    wd2_d = nc.dram_tensor("wd2bc", (128, DD), f32, kind="ExternalInput").ap()
    bd2_d = nc.dram_tensor("bd2c", (128, 1), f32, kind="ExternalInput").ap()
    outp_d = nc.dram_tensor("outp", (TS, D), f32, kind="ExternalOutput").ap()
    usage_d = nc.dram_tensor("usage", (1, E), f32, kind="ExternalOutput").ap()

    NSUB = TS // 128  # 4 t-subtiles per core

    with tile.TileContext(nc) as tc:
        with ExitStack() as ctx:
            cp = ctx.enter_context(tc.tile_pool(name="const", bufs=1))
            w2p = ctx.enter_context(tc.tile_pool(name="w2res", bufs=1))
            w1p = ctx.enter_context(tc.tile_pool(name="w1s", bufs=4))
            hp = ctx.enter_context(tc.tile_pool(name="hd", bufs=2))
            op = ctx.enter_context(tc.tile_pool(name="otmp", bufs=3))
            pmm = ctx.enter_context(tc.tile_pool(name="pmm", bufs=2, space="PSUM"))
            ph = ctx.enter_context(tc.tile_pool(name="ph", bufs=2, space="PSUM"))
            pl = ctx.enter_context(tc.tile_pool(name="pl", bufs=1, space="PSUM"))
            pu = ctx.enter_context(tc.tile_pool(name="pu", bufs=1, space="PSUM"))

            # ---- resident tiles ----
            xtf = cp.tile([128, KX, TS], f32)       # x^T shard, fp32 (router)
            xtr = cp.tile([128, KX - 1, TS], f32r)  # x^T shard, f32r (experts)
            wd1t = cp.tile([128, KX, DD], f32)
            wgt = cp.tile([128, KX, E], f32)
            b1t = cp.tile([128, E, F // 128], f32)
            wd2t = cp.tile([128, DD], f32)
            bd2t = cp.tile([128, 1], f32)
            h1T = cp.tile([128, KF, TS], f32r)      # relu(x@W1+b1)^T + aug row
            acc = cp.tile([128, NSUB, D], f32)      # combined output accumulator
            Lg = cp.tile([128, NSUB, E], f32)       # logits, token-major
            zc = cp.tile([128, NSUB, 1], f32)       # pre-softplus difficulty
            uacc = cp.tile([128, NSUB, E], f32)     # usage accumulator
            onesc = cp.tile([128, 1], f32)
            Rk = cp.tile([128, NSUB, E], f32)       # rank
            cj = cp.tile([128, NSUB, E], f32)
            Am = cp.tile([128, NSUB, E], f32)       # active mask
            Ex = cp.tile([128, NSUB, E], f32)       # exp(l - max)
            Em = cp.tile([128, NSUB, E], f32)
            wmat = cp.tile([128, NSUB, E], f32)     # routing weights
            nrm = cp.tile([128, NSUB, 1], f32)
            s8 = cp.tile([128, NSUB, 1], f32)
            sm = cp.tile([128, NSUB, 1], f32)
            rm = cp.tile([128, NSUB, 1], f32)
            r8 = cp.tile([128, NSUB, 1], f32)
            ff = cp.tile([128, NSUB, 1], f32)
            kfl = cp.tile([128, NSUB], f32)
            kint = cp.tile([128, NSUB], i32)
            ent = cp.tile([128, NSUB], f32)
            e1 = cp.tile([128, NSUB], f32)
            uw = cp.tile([128, NSUB, E], f32)
            usg = cp.tile([1, E], f32)
            u1 = cp.tile([1, E], f32)
            u2 = cp.tile([1, E], f32)

            # ---- loads + init ----
            nc.sync.dma_start(xtf[:], xs_d[:])
            nc.sync.dma_start(wd1t[:], wd1_d[:])
            nc.sync.dma_start(wgt[:], wg_d[:])
            nc.sync.dma_start(b1t[:], b1_d[:])
            nc.sync.dma_start(wd2t[:], wd2_d[:])
            nc.sync.dma_start(bd2t[:], bd2_d[:])
            nc.vector.tensor_copy(xtr[:], xtf[:, : KX - 1, :])  # fp32 -> f32r
            augsrc = cp.tile([128, TS], f32)
            nc.vector.memset(augsrc[:], 0.0)
            nc.vector.memset(augsrc[0:1, :], 1.0)
            nc.vector.tensor_copy(h1T[:, KF - 1, :], augsrc[:])
            nc.vector.memset(onesc[:], 1.0)
            nc.vector.memset(uacc[:], 0.0)

            # ---- router phase (exact fp32) ----
            for s in range(NSUB):
                psumh = ph.tile([128, DD], f32, tag="ph")
                for k in range(KX):
                    nc.tensor.matmul(
                        psumh[:],
                        xtf[:, k, s * 128 : (s + 1) * 128],
                        wd1t[:, k, :],
                        start=(k == 0),
                        stop=(k == KX - 1),
                    )
                psuml = pl.tile([128, E], f32, tag="pl")
                for k in range(KX):
                    nc.tensor.matmul(
                        psuml[:],
                        xtf[:, k, s * 128 : (s + 1) * 128],
                        wgt[:, k, :],
                        start=(k == 0),
                        stop=(k == KX - 1),
                    )
                nc.vector.tensor_copy(Lg[:, s, :], psuml[:])
                hds = hp.tile([128, DD], f32, tag="hds")
                nc.scalar.activation(hds[:], psumh[:], AF.Relu)
                scr = hp.tile([128, DD], f32, tag="scr")
                nc.vector.tensor_tensor(scr[:], hds[:], wd2t[:], ALU.mult)
                nc.vector.tensor_reduce(zc[:, s, :], scr[:], axis=AX.X, op=ALU.add)

            # entropy -> k
            nc.scalar.activation(e1[:], zc[:, :, 0], AF.Exp, bias=bd2t[:])
            nc.vector.tensor_scalar_add(e1[:], e1[:], 1.0)
            nc.scalar.activation(ent[:], e1[:], AF.Ln)
            nc.vector.tensor_scalar(
                ent[:], ent[:], -T_LOW, 1.0 / (T_HIGH - T_LOW), ALU.add, ALU.mult
            )
            nc.vector.tensor_scalar(ent[:], ent[:], 0.0, 1.0, ALU.max, ALU.min)
            nc.vector.tensor_scalar(
                ent[:], ent[:], float(MAX_E - MIN_E), float(MIN_E), ALU.mult, ALU.add
            )
            nc.vector.tensor_copy(kint[:], ent[:])   # rint (half-to-even)
            nc.vector.tensor_copy(kfl[:], kint[:])

            # rank + active mask
            sh = (128, NSUB, E)
            for j in range(E):
                colb = Lg[:, :, j : j + 1].to_broadcast(sh)
                if j == 0:
                    nc.vector.tensor_tensor(Rk[:], Lg[:], colb, ALU.is_lt)
                else:
                    nc.vector.tensor_tensor(cj[:], Lg[:], colb, ALU.is_lt)
                    nc.vector.tensor_tensor(Rk[:], Rk[:], cj[:], ALU.add)
            kb = kfl[:, :, None].to_broadcast(sh)
            nc.vector.tensor_tensor(Am[:], Rk[:], kb, ALU.is_lt)

            # softmaxes (full for usage, masked for weights)
            nc.vector.tensor_reduce(nrm[:, :, 0], Lg[:], axis=AX.X, op=ALU.max, negate=True)
            nc.vector.tensor_tensor(Ex[:], Lg[:], nrm[:].to_broadcast(sh), ALU.add)
            nc.scalar.activation(Ex[:], Ex[:], AF.Exp)
            nc.vector.tensor_reduce(s8[:, :, 0], Ex[:], axis=AX.X, op=ALU.add)
            nc.vector.tensor_tensor(Em[:], Ex[:], Am[:], ALU.mult)
            nc.vector.tensor_reduce(sm[:, :, 0], Em[:], axis=AX.X, op=ALU.add)
            nc.vector.reciprocal(rm[:], sm[:])
            nc.vector.tensor_tensor(wmat[:], Em[:], rm[:].to_broadcast(sh), ALU.mult)

            # usage partial: sum_t softmax(l) * k/8
            nc.vector.reciprocal(r8[:], s8[:])
            nc.vector.tensor_tensor(ff[:], kfl[:, :, None], r8[:], ALU.mult)
            nc.vector.tensor_scalar_mul(ff[:], ff[:], 1.0 / MAX_E)
            nc.vector.tensor_tensor(uw[:], Ex[:], ff[:].to_broadcast(sh), ALU.mult)
            nc.vector.tensor_tensor(uacc[:], uacc[:], uw[:], ALU.add)
            psu = pu.tile([1, NSUB, E], f32)
            nc.tensor.matmul(
                psu[:1, :, :].rearrange("p a b -> p (a b)"),
                onesc[:],
                uacc[:].rearrange("p a b -> p (a b)"),
                start=True,
                stop=True,
            )
            pusb = cp.tile([1, NSUB, E], f32)
            nc.vector.tensor_copy(pusb[:], psu[:1, :, :])
            nc.vector.tensor_tensor(u1[:], pusb[:1, 0, :], pusb[:1, 1, :], ALU.add)
            nc.vector.tensor_tensor(u2[:], pusb[:1, 2, :], pusb[:1, 3, :], ALU.add)
            nc.vector.tensor_tensor(usg[:], u1[:], u2[:], ALU.add)
            nc.sync.dma_start(usage_d[:], usg[:])

            # ---- expert loop ----
            for e in range(E):
                w2res = w2p.tile([128, KF, D], f32r)
                nc.sync.dma_start(
                    w2res[:], w2_d[e].rearrange("(ko p) d -> p ko d", p=128)
                )
                # mm1: h1T = relu(W1^T x + b1)
                for m in range(F // 128):
                    w1t = w1p.tile([128, KX - 1, 128], f32r)
                    nc.sync.dma_start(
                        w1t[:],
                        w1_d[e].rearrange("(ko p) f -> p ko f", p=128)[
                            :, :, m * 128 : (m + 1) * 128
                        ],
                    )
                    psum1 = pmm.tile([128, TS], f32, tag="pmm")
                    for k in range(KX - 1):
                        nc.tensor.matmul(
                            psum1[:],
                            w1t[:, k, :],
                            xtr[:, k, :],
                            start=(k == 0),
                            stop=(k == KX - 2),
                        )
                    nc.scalar.activation(
                        h1T[:, m, :], psum1[:], AF.Relu, bias=b1t[:, e, m : m + 1]
                    )
                # mm2 + weighted accumulate
                for mt in range(NSUB):
                    for dn in range(2):
                        psum2 = pmm.tile([128, DD], f32, tag="pmm")
                        for k in range(KF):
                            nc.tensor.matmul(
                                psum2[:],
                                h1T[:, k, mt * 128 : (mt + 1) * 128],
                                w2res[:, k, dn * DD : (dn + 1) * DD],
                                start=(k == 0),
                                stop=(k == KF - 1),
                            )
                        wcol = wmat[:, mt, e : e + 1]
                        dst = acc[:, mt, dn * DD : (dn + 1) * DD]
                        if e == 0:
                            nc.scalar.activation(dst, psum2[:], AF.Identity, scale=wcol)
                        else:
                            otmp = op.tile([128, DD], f32)
                            nc.scalar.activation(
                                otmp[:], psum2[:], AF.Identity, scale=wcol
                            )
                            nc.vector.tensor_tensor(dst, dst, otmp[:], ALU.add)

            # ---- store ----
            for mt in range(NSUB):
                nc.sync.dma_start(
                    outp_d[mt * 128 : (mt + 1) * 128, :], acc[:, mt, :]
                )

    nc.compile()
    return nc


def _get_program():
    if "nc" not in _prog_cache:
        _prog_cache["nc"] = _build_program()
    return _prog_cache["nc"]


def kernel(x, W1, b1, W2, b2, Wg, bg, Wd1, bd1, Wd2, bd2):
    from concourse.bass_utils import run_bass_kernel_spmd

    x = np.asarray(x, dtype=np.float32)
    W1 = np.ascontiguousarray(np.asarray(W1, dtype=np.float32))
    b1 = np.asarray(b1, dtype=np.float32)
    W2 = np.asarray(W2, dtype=np.float32)
    b2 = np.asarray(b2, dtype=np.float32)
    Wg = np.asarray(Wg, dtype=np.float32)
    bg = np.asarray(bg, dtype=np.float32)
    Wd1 = np.asarray(Wd1, dtype=np.float32)
    bd1 = np.asarray(bd1, dtype=np.float32)
    Wd2 = np.asarray(Wd2, dtype=np.float32)
    bd2 = np.asarray(bd2, dtype=np.float32)

    nc = _get_program()

    xf = x.reshape(T, D)
    xaug = np.zeros((KX * 128, T), dtype=np.float32)
    xaug[:D] = xf.T
    xaug[D] = 1.0

    w2a = np.zeros((E, KF * 128, D), dtype=np.float32)
    w2a[:, :F] = W2
    w2a[:, F] = b2

    wd1a = np.zeros((KX * 128, DD), dtype=np.float32)
    wd1a[:D] = Wd1
    wd1a[D] = bd1
    wd1a3 = np.ascontiguousarray(
        wd1a.reshape(KX, 128, DD).transpose(1, 0, 2)
    )

    wga = np.zeros((KX * 128, E), dtype=np.float32)
    wga[:D] = Wg
    wga[D] = bg
    wga3 = np.ascontiguousarray(wga.reshape(KX, 128, E).transpose(1, 0, 2))

    b1p = np.ascontiguousarray(
        b1.reshape(E, F // 128, 128).transpose(2, 0, 1)
    )  # [p, e, m] = b1[e, m*128+p]
    wd2bc = np.ascontiguousarray(np.broadcast_to(Wd2[:, 0][None, :], (128, DD)))
    bd2c = np.full((128, 1), bd2[0], dtype=np.float32)

    in_maps = []
    for c in range(NC):
        xs3 = np.ascontiguousarray(
            xaug[:, c * TS : (c + 1) * TS].reshape(KX, 128, TS).transpose(1, 0, 2)
        )
        in_maps.append(
            {
                "xs": xs3,
                "w1": W1,
                "w2a": w2a,
                "wd1a": wd1a3,
                "wga": wga3,
                "b1p": b1p,
                "wd2bc": wd2bc,
                "bd2c": bd2c,
            }
        )

    global _last_in_maps
    _last_in_maps = in_maps
    res = run_bass_kernel_spmd(nc, in_maps, core_ids=list(range(NC)))

    out = np.concatenate([res.results[c]["outp"] for c in range(NC)], axis=0)
    usage = np.sum(
        np.stack([res.results[c]["usage"][0] for c in range(NC)]), axis=0
    ).astype(np.float32)
    mean = np.float32(usage.mean())
    std = np.float32(usage.std(ddof=1))
    cv = std / (mean + np.float32(1e-6))
    return out.reshape(B, N, D), np.float32(cv)


# revision 11
# speedup vs baseline: 1.2125x; 1.2125x over previous
"""Trainium2 Bass kernel for ConditionalMoEWithLoadBalancing.

Sharding: token-parallel SPMD over 8 NeuronCores. Core c processes tokens
[c*512, (c+1)*512) through the full network (router + difficulty net in exact
fp32, all 8 expert FFNs in float32r). Expert weights are streamed from HBM
per expert; the weighted combine happens on-chip, so the host only
concatenates the 8 output shards and sums the 8 usage partials for the cv
scalar.

Self-contained: hardcodes all shapes for the fixed problem instance
(x (2,2048,1024), E=8, D=1024).
"""

import numpy as np

# problem dims
B, N, D = 2, 2048, 1024
T = B * N            # 4096 tokens
E = 8
F = 2 * D            # 2048
DD = D // 2          # 512
NC = 8               # cores
TS = T // NC         # 512 tokens per core
KX = D // 128 + 1    # 9 k-tiles for x (aug bias row)
KF = F // 128 + 1    # 17 k-tiles for h1 (aug bias row)
MIN_E, MAX_E = 1, 8
T_LOW, T_HIGH = 0.5, 2.0

_prog_cache = {}


def _build_program():
    import concourse.tile as tile
    import concourse.mybir as mybir
    from concourse import bacc
    from contextlib import ExitStack

    f32 = mybir.dt.float32
    f32r = mybir.dt.float32r
    f16 = mybir.dt.float16
    i32 = mybir.dt.int32
    AF = mybir.ActivationFunctionType
    ALU = mybir.AluOpType
    AX = mybir.AxisListType

    nc = bacc.Bacc(
        "TRN2",
        target_bir_lowering=False,
        debug=False,
        enable_asserts=False,
        num_devices=NC,
    )

    # ---- DRAM I/O ----
    xs_d = nc.dram_tensor("xs", (128, KX, TS), f32, kind="ExternalInput").ap()
    w1_d = nc.dram_tensor("w1", (E, D, F), f16, kind="ExternalInput").ap()
    w2_d = nc.dram_tensor("w2a", (E, KF * 128, D), f16, kind="ExternalInput").ap()
    wd1_d = nc.dram_tensor("wd1a", (128, KX, DD), f32, kind="ExternalInput").ap()
    wg_d = nc.dram_tensor("wga", (128, KX, E), f32, kind="ExternalInput").ap()
    b1_d = nc.dram_tensor("b1p", (128, E, F // 128), f32, kind="ExternalInput").ap()
    wd2_d = nc.dram_tensor("wd2bc", (128, DD), f32, kind="ExternalInput").ap()
    bd2_d = nc.dram_tensor("bd2c", (128, 1), f32, kind="ExternalInput").ap()
    outp_d = nc.dram_tensor("outp", (TS, D), f32, kind="ExternalOutput").ap()
    usage_d = nc.dram_tensor("usage", (1, E), f32, kind="ExternalOutput").ap()

    NSUB = TS // 128  # 4 t-subtiles per core

    with tile.TileContext(nc) as tc:
        with ExitStack() as ctx:
            cp = ctx.enter_context(tc.tile_pool(name="const", bufs=1))
            w2p = ctx.enter_context(tc.tile_pool(name="w2res", bufs=2))
            w1p = ctx.enter_context(tc.tile_pool(name="w1s", bufs=4))
            hp = ctx.enter_context(tc.tile_pool(name="hd", bufs=2))
            op = ctx.enter_context(tc.tile_pool(name="otmp", bufs=3))
            pmm = ctx.enter_context(tc.tile_pool(name="pmm", bufs=2, space="PSUM"))
            ph = ctx.enter_context(tc.tile_pool(name="ph", bufs=2, space="PSUM"))
            pl = ctx.enter_context(tc.tile_pool(name="pl", bufs=1, space="PSUM"))
            pu = ctx.enter_context(tc.tile_pool(name="pu", bufs=1, space="PSUM"))

            # ---- resident tiles ----
            xtf = cp.tile([128, KX, TS], f32)       # x^T shard, fp32 (router)
            xtr = cp.tile([128, KX - 1, TS], f16)   # x^T shard, fp16 (experts)
            wd1t = cp.tile([128, KX, DD], f32)
            wgt = cp.tile([128, KX, E], f32)
            b1t = cp.tile([128, E, F // 128], f32)
            wd2t = cp.tile([128, DD], f32)
            bd2t = cp.tile([128, 1], f32)
            h1T = cp.tile([128, KF, TS], f16)       # relu(x@W1+b1)^T + aug row
            acc = cp.tile([128, NSUB, D], f32)      # combined output accumulator
            Lg = cp.tile([128, NSUB, E], f32)       # logits, token-major
            zc = cp.tile([128, NSUB, 1], f32)       # pre-softplus difficulty
            uacc = cp.tile([128, NSUB, E], f32)     # usage accumulator
            onesc = cp.tile([128, 1], f32)
            Rk = cp.tile([128, NSUB, E], f32)       # rank
            cj = cp.tile([128, NSUB, E], f32)
            Am = cp.tile([128, NSUB, E], f32)       # active mask
            Ex = cp.tile([128, NSUB, E], f32)       # exp(l - max)
            Em = cp.tile([128, NSUB, E], f32)
            wmat = cp.tile([128, NSUB, E], f32)     # routing weights
            nrm = cp.tile([128, NSUB, 1], f32)
            s8 = cp.tile([128, NSUB, 1], f32)
            sm = cp.tile([128, NSUB, 1], f32)
            rm = cp.tile([128, NSUB, 1], f32)
            r8 = cp.tile([128, NSUB, 1], f32)
            ff = cp.tile([128, NSUB, 1], f32)
            kfl = cp.tile([128, NSUB], f32)
            kint = cp.tile([128, NSUB], i32)
            ent = cp.tile([128, NSUB], f32)
            e1 = cp.tile([128, NSUB], f32)
            uw = cp.tile([128, NSUB, E], f32)
            usg = cp.tile([1, E], f32)
            u1 = cp.tile([1, E], f32)
            u2 = cp.tile([1, E], f32)

            # ---- loads + init ----
            nc.sync.dma_start(xtf[:], xs_d[:])
            nc.sync.dma_start(wd1t[:], wd1_d[:])
            nc.sync.dma_start(wgt[:], wg_d[:])
            nc.sync.dma_start(b1t[:], b1_d[:])
            nc.sync.dma_start(wd2t[:], wd2_d[:])
            nc.sync.dma_start(bd2t[:], bd2_d[:])
            nc.vector.tensor_copy(xtr[:], xtf[:, : KX - 1, :])  # fp32 -> fp16
            augsrc = cp.tile([128, TS], f32)
            nc.vector.memset(augsrc[:], 0.0)
            nc.vector.memset(augsrc[0:1, :], 1.0)
            nc.vector.tensor_copy(h1T[:, KF - 1, :], augsrc[:])
            nc.vector.memset(onesc[:], 1.0)
            nc.vector.memset(uacc[:], 0.0)

            # ---- router phase (exact fp32) ----
            for s in range(NSUB):
                psumh = ph.tile([128, DD], f32, tag="ph")
                for k in range(KX):
                    nc.tensor.matmul(
                        psumh[:],
                        xtf[:, k, s * 128 : (s + 1) * 128],
                        wd1t[:, k, :],
                        start=(k == 0),
                        stop=(k == KX - 1),
                    )
                psuml = pl.tile([128, E], f32, tag="pl")
                for k in range(KX):
                    nc.tensor.matmul(
                        psuml[:],
                        xtf[:, k, s * 128 : (s + 1) * 128],
                        wgt[:, k, :],
                        start=(k == 0),
                        stop=(k == KX - 1),
                    )
                nc.vector.tensor_copy(Lg[:, s, :], psuml[:])
                hds = hp.tile([128, DD], f32, tag="hds")
                nc.scalar.activation(hds[:], psumh[:], AF.Relu)
                scr = hp.tile([128, DD], f32, tag="scr")
                nc.vector.tensor_tensor(scr[:], hds[:], wd2t[:], ALU.mult)
                nc.vector.tensor_reduce(zc[:, s, :], scr[:], axis=AX.X, op=ALU.add)

            # entropy -> k
            nc.scalar.activation(e1[:], zc[:, :, 0], AF.Exp, bias=bd2t[:])
            nc.vector.tensor_scalar_add(e1[:], e1[:], 1.0)
            nc.scalar.activation(ent[:], e1[:], AF.Ln)
            nc.vector.tensor_scalar(
                ent[:], ent[:], -T_LOW, 1.0 / (T_HIGH - T_LOW), ALU.add, ALU.mult
            )
            nc.vector.tensor_scalar(ent[:], ent[:], 0.0, 1.0, ALU.max, ALU.min)
            nc.vector.tensor_scalar(
                ent[:], ent[:], float(MAX_E - MIN_E), float(MIN_E), ALU.mult, ALU.add
            )
            nc.vector.tensor_copy(kint[:], ent[:])   # rint (half-to-even)
            nc.vector.tensor_copy(kfl[:], kint[:])

            # rank + active mask
            sh = (128, NSUB, E)
            for j in range(E):
                colb = Lg[:, :, j : j + 1].to_broadcast(sh)
                if j == 0:
                    nc.vector.tensor_tensor(Rk[:], Lg[:], colb, ALU.is_lt)
                else:
                    nc.vector.tensor_tensor(cj[:], Lg[:], colb, ALU.is_lt)
                    nc.vector.tensor_tensor(Rk[:], Rk[:], cj[:], ALU.add)
            kb = kfl[:, :, None].to_broadcast(sh)
            nc.vector.tensor_tensor(Am[:], Rk[:], kb, ALU.is_lt)

            # softmaxes (full for usage, masked for weights)
            nc.vector.tensor_reduce(nrm[:, :, 0], Lg[:], axis=AX.X, op=ALU.max, negate=True)
            nc.vector.tensor_tensor(Ex[:], Lg[:], nrm[:].to_broadcast(sh), ALU.add)
            nc.scalar.activation(Ex[:], Ex[:], AF.Exp)
            nc.vector.tensor_reduce(s8[:, :, 0], Ex[:], axis=AX.X, op=ALU.add)
            nc.vector.tensor_tensor(Em[:], Ex[:], Am[:], ALU.mult)
            nc.vector.tensor_reduce(sm[:, :, 0], Em[:], axis=AX.X, op=ALU.add)
            nc.vector.reciprocal(rm[:], sm[:])
            nc.vector.tensor_tensor(wmat[:], Em[:], rm[:].to_broadcast(sh), ALU.mult)

            # usage partial: sum_t softmax(l) * k/8
            nc.vector.reciprocal(r8[:], s8[:])
            nc.vector.tensor_tensor(ff[:], kfl[:, :, None], r8[:], ALU.mult)
            nc.vector.tensor_scalar_mul(ff[:], ff[:], 1.0 / MAX_E)
            nc.vector.tensor_tensor(uw[:], Ex[:], ff[:].to_broadcast(sh), ALU.mult)
            nc.vector.tensor_tensor(uacc[:], uacc[:], uw[:], ALU.add)
            psu = pu.tile([1, NSUB, E], f32)
            nc.tensor.matmul(
                psu[:1, :, :].rearrange("p a b -> p (a b)"),
                onesc[:],
                uacc[:].rearrange("p a b -> p (a b)"),
                start=True,
                stop=True,
            )
            pusb = cp.tile([1, NSUB, E], f32)
            nc.vector.tensor_copy(pusb[:], psu[:1, :, :])
            nc.vector.tensor_tensor(u1[:], pusb[:1, 0, :], pusb[:1, 1, :], ALU.add)
            nc.vector.tensor_tensor(u2[:], pusb[:1, 2, :], pusb[:1, 3, :], ALU.add)
            nc.vector.tensor_tensor(usg[:], u1[:], u2[:], ALU.add)
            nc.sync.dma_start(usage_d[:], usg[:])

            # ---- expert loop ----
            for e in range(E):
                w2res = w2p.tile([128, KF, D], f16)
                nc.sync.dma_start(
                    w2res[:], w2_d[e].rearrange("(ko p) d -> p ko d", p=128)
                )
                # mm1: h1T = relu(W1^T x + b1)
                for m in range(F // 128):
                    w1t = w1p.tile([128, KX - 1, 128], f16)
                    nc.sync.dma_start(
                        w1t[:],
                        w1_d[e].rearrange("(ko p) f -> p ko f", p=128)[
                            :, :, m * 128 : (m + 1) * 128
                        ],
                    )
                    psum1 = pmm.tile([128, TS], f32, tag="pmm")
                    for k in range(KX - 1):
                        nc.tensor.matmul(
                            psum1[:],
                            w1t[:, k, :],
                            xtr[:, k, :],
                            start=(k == 0),
                            stop=(k == KX - 2),
                        )
                    nc.scalar.activation(
                        h1T[:, m, :], psum1[:], AF.Relu, bias=b1t[:, e, m : m + 1]
                    )
                # mm2 + weighted accumulate
                for mt in range(NSUB):
                    for dn in range(2):
                        psum2 = pmm.tile([128, DD], f32, tag="pmm")
                        for k in range(KF):
                            nc.tensor.matmul(
                                psum2[:],
                                h1T[:, k, mt * 128 : (mt + 1) * 128],
                                w2res[:, k, dn * DD : (dn + 1) * DD],
                                start=(k == 0),
                                stop=(k == KF - 1),
                            )
                        wcol = wmat[:, mt, e : e + 1]
                        dst = acc[:, mt, dn * DD : (dn + 1) * DD]
                        if e == 0:
                            nc.scalar.activation(dst, psum2[:], AF.Identity, scale=wcol)
                        else:
                            otmp = op.tile([128, DD], f32)
                            nc.scalar.activation(
                                otmp[:], psum2[:], AF.Identity, scale=wcol
                            )
                            nc.vector.tensor_tensor(dst, dst, otmp[:], ALU.add)

            # ---- store ----
            for mt in range(NSUB):
                nc.sync.dma_start(
                    outp_d[mt * 128 : (mt + 1) * 128, :], acc[:, mt, :]
                )

    nc.compile()
    return nc


def _get_program():
    if "nc" not in _prog_cache:
        _prog_cache["nc"] = _build_program()
    return _prog_cache["nc"]


def kernel(x, W1, b1, W2, b2, Wg, bg, Wd1, bd1, Wd2, bd2):
    from concourse.bass_utils import run_bass_kernel_spmd

    x = np.asarray(x, dtype=np.float32)
    W1 = np.ascontiguousarray(np.asarray(W1, dtype=np.float32))
    b1 = np.asarray(b1, dtype=np.float32)
    W2 = np.asarray(W2, dtype=np.float32)
    b2 = np.asarray(b2, dtype=np.float32)
    Wg = np.asarray(Wg, dtype=np.float32)
    bg = np.asarray(bg, dtype=np.float32)
    Wd1 = np.asarray(Wd1, dtype=np.float32)
    bd1 = np.asarray(bd1, dtype=np.float32)
    Wd2 = np.asarray(Wd2, dtype=np.float32)
    bd2 = np.asarray(bd2, dtype=np.float32)

    nc = _get_program()

    xf = x.reshape(T, D)
    xaug = np.zeros((KX * 128, T), dtype=np.float32)
    xaug[:D] = xf.T
    xaug[D] = 1.0

    w2a = np.zeros((E, KF * 128, D), dtype=np.float32)
    w2a[:, :F] = W2
    w2a[:, F] = b2
    w2ah = w2a.astype(np.float16)
    W1h = W1.astype(np.float16)

    wd1a = np.zeros((KX * 128, DD), dtype=np.float32)
    wd1a[:D] = Wd1
    wd1a[D] = bd1
    wd1a3 = np.ascontiguousarray(
        wd1a.reshape(KX, 128, DD).transpose(1, 0, 2)
    )

    wga = np.zeros((KX * 128, E), dtype=np.float32)
    wga[:D] = Wg
    wga[D] = bg
    wga3 = np.ascontiguousarray(wga.reshape(KX, 128, E).transpose(1, 0, 2))

    b1p = np.ascontiguousarray(
        b1.reshape(E, F // 128, 128).transpose(2, 0, 1)
    )  # [p, e, m] = b1[e, m*128+p]
    wd2bc = np.ascontiguousarray(np.broadcast_to(Wd2[:, 0][None, :], (128, DD)))
    bd2c = np.full((128, 1), bd2[0], dtype=np.float32)

    in_maps = []
    for c in range(NC):
        xs3 = np.ascontiguousarray(
            xaug[:, c * TS : (c + 1) * TS].reshape(KX, 128, TS).transpose(1, 0, 2)
        )
        in_maps.append(
            {
                "xs": xs3,
                "w1": W1h,
                "w2a": w2ah,
                "wd1a": wd1a3,
                "wga": wga3,
                "b1p": b1p,
                "wd2bc": wd2bc,
                "bd2c": bd2c,
            }
        )

    global _last_in_maps
    _last_in_maps = in_maps
    res = run_bass_kernel_spmd(nc, in_maps, core_ids=list(range(NC)))

    out = np.concatenate([res.results[c]["outp"] for c in range(NC)], axis=0)
    usage = np.sum(
        np.stack([res.results[c]["usage"][0] for c in range(NC)]), axis=0
    ).astype(np.float32)
    mean = np.float32(usage.mean())
    std = np.float32(usage.std(ddof=1))
    cv = std / (mean + np.float32(1e-6))
    return out.reshape(B, N, D), np.float32(cv)
